# revision 1
# baseline (speedup 1.0000x reference)
import sys

sys.path.insert(0, "/opt/trn_rl_repo")

import numpy as np

H, W = 30, 30
D = 256
K = 16
HW = H * W
SENT = HW
B = 128
NCORES = 8
S = B // NCORES
SO = S * K
PCHUNK = 1024



def _label_components(grid):
    lin = np.arange(HW, dtype=np.int32).reshape(1, H, W)
    fg = grid > 0
    lab = np.where(fg, lin, SENT).astype(np.int32)
    gp = np.pad(grid, ((0, 0), (1, 1), (1, 1)), constant_values=-1)
    nb = grid.shape[0]
    while True:
        lp = np.pad(lab, ((0, 0), (1, 1), (1, 1)), constant_values=SENT)
        m = lab.copy()
        for di, dj in ((-1, 0), (1, 0), (0, -1), (0, 1)):
            ls = lp[:, 1 + di:1 + di + H, 1 + dj:1 + dj + W]
            gs = gp[:, 1 + di:1 + di + H, 1 + dj:1 + dj + W]
            m = np.minimum(m, np.where(gs == grid, ls, SENT))
        m = np.where(fg, m, SENT)
        flat = m.reshape(nb, HW)
        jumped = np.take_along_axis(flat, np.clip(flat, 0, HW - 1), axis=1)
        flat = np.where(flat < SENT, np.minimum(flat, jumped), SENT)
        new = flat.reshape(nb, H, W)
        if np.array_equal(new, lab):
            return new
        lab = new


def _build_masks(grid):
    nb = grid.shape[0]
    labels = _label_components(grid).reshape(nb, HW)
    gf = grid.reshape(nb, HW)
    lin = np.arange(HW, dtype=np.int32)
    rows, cols = lin // W, lin % W
    mhat = np.zeros((nb, HW, K), np.float32)
    bboxT = np.zeros((nb, 5, K), np.float32)
    validf = np.zeros((nb, K), np.float32)
    for b in range(nb):
        l = labels[b]
        roots = np.nonzero((l == lin) & (l < SENT))[0][:K]
        for k, r in enumerate(roots):
            memb = l == r
            rs, cs = rows[memb], cols[memb]
            y, x = int(rs.min()), int(cs.min())
            h = int(rs.max()) + 1 - y
            w = int(cs.max()) + 1 - x
            inb = ((rows >= y) & (rows < y + h) & (cols >= x) & (cols < x + w))
            mhat[b, :HW, k] = inb.astype(np.float32) / float(h * w)
            bboxT[b, :, k] = (gf[b, r] / 9.0, x / float(W), y / float(H),
                              w / float(W), h / float(H))
            validf[b, k] = 1.0
    return mhat, bboxT, validf


def _row_cap(mhat):
    nz = np.nonzero(np.abs(mhat).sum(axis=(0, 2)) > 0)[0]
    need = int(nz.max()) + 1 if len(nz) else 1
    return max(128, -(-need // 128) * 128)



_PROG = {}
GRP = 4


def _build_program(rcap):
    import concourse.bacc as bacc
    import concourse.mybir as mybir
    import concourse.tile as tile

    f32 = mybir.dt.float32
    nch = rcap // 128
    nc = bacc.Bacc("TRN2", target_bir_lowering=False, debug=False,
                   num_devices=NCORES)

    ge = nc.declare_dram_parameter("ge", [S * HW, D], f32, isOutput=False)
    mh = nc.declare_dram_parameter("mh", [S * rcap, K], f32, isOutput=False)
    wall = nc.declare_dram_parameter("wall", [128, 6 * D], f32, isOutput=False)
    w1c = nc.declare_dram_parameter("w1c", [5, D], f32, isOutput=False)
    bpk = nc.declare_dram_parameter("bpk", [128, 11], f32, isOutput=False)
    spk = nc.declare_dram_parameter("spk", [128, 400], f32, isOutput=False)
    epk = nc.declare_dram_parameter("epk", [S, 768], f32, isOutput=False)
    out = nc.declare_dram_parameter("out", [SO, D], f32, isOutput=True)

    AF = mybir.ActivationFunctionType
    MUL = mybir.AluOpType.mult
    ADD = mybir.AluOpType.add

    with tile.TileContext(nc) as tc:
        with (
            tc.tile_pool(name="const", bufs=1) as cpool,
            tc.tile_pool(name="gp", bufs=2) as gpool,
            tc.tile_pool(name="act", bufs=1) as apool,
            tc.tile_pool(name="scr", bufs=2) as spool,
            tc.tile_pool(name="ps", bufs=2, space="PSUM") as pspool,
            tc.tile_pool(name="pp", bufs=1, space="PSUM") as pppool,
        ):
            wallt = cpool.tile([128, 6 * D], f32, tag="wall", name="wall")
            nc.scalar.dma_start(wallt[:], wall[:])
            w1ct = cpool.tile([5, D], f32, tag="w1c", name="w1c")
            nc.scalar.dma_start(w1ct[:], w1c[:])
            bpkt = cpool.tile([128, 11], f32, tag="bpk", name="bpk")
            nc.scalar.dma_start(bpkt[:], bpk[:])
            spkt = cpool.tile([128, 400], f32, tag="spk", name="spk")
            nc.scalar.dma_start(spkt[:], spk[:])
            epkt = cpool.tile([S, 768], f32, tag="epk", name="epk")
            nc.scalar.dma_start(epkt[:], epk[:])
            mtall = cpool.tile([128, S * nch * K], f32, tag="mh", name="mh")
            nc.scalar.dma_start(
                mtall[:],
                mh.rearrange("(s c p) k -> p s c k", s=S, c=nch, p=128))

            w1t = [wallt[:, 0:256], wallt[:, 256:512], w1ct[:]]
            w2t = [wallt[:, 512:768], wallt[:, 768:1024]]
            wpt = [wallt[:, 1024:1280], wallt[:, 1280:1536]]

            def bias_ap(j, m):
                return bpkt[:, m * 5 + j:m * 5 + j + 1]

            orthob = bpkt[:, 10:11]
            selt = spkt[:, 0:16]
            srt = spkt[:, 16:272]
            idt = spkt[:, 272:400]
            et = epkt[:, 0:256]
            vldt = epkt[0:1, 256:512]
            bbxt = epkt[0:5, 512:768]

            onescol = cpool.tile([128, 1], f32, tag="onescol", name="onescol")
            nc.vector.memset(onescol[:], 1.0)
            onesrow = cpool.tile([1, 128], f32, tag="onesrow", name="onesrow")
            nc.vector.memset(onesrow[:], 1.0)

            def bcast_row(row_ap, tag):
                pb = pspool.tile([128, SO], f32, tag="big", name=f"bc_{tag}")
                nc.tensor.matmul(pb[:], onesrow[:], row_ap, start=True,
                                 stop=True)
                sb = spool.tile([128, SO], f32, tag=f"bcs_{tag}",
                                name=f"bcs_{tag}")
                nc.vector.tensor_copy(sb[:], pb[:])
                return sb

            validb = bcast_row(vldt, "vld")

            psn = pspool.tile([S, D], f32, tag="row", name="ssum")
            nc.tensor.matmul(psn[:], selt, srt, start=True, stop=True)
            st = spool.tile([S, D], f32, tag="smean", name="smean")
            nc.vector.tensor_scalar_mul(st[:], psn[:], 0.125)
            sq = spool.tile([S, D], f32, tag="ssq", name="ssq")
            nc.vector.tensor_mul(sq[:], st[:], st[:])
            rs = spool.tile([S, 1], f32, tag="srs", name="srs")
            nc.vector.reduce_sum(rs[:], sq[:], axis=mybir.AxisListType.X)
            nc.vector.tensor_scalar_max(rs[:], rs[:], 1e-16)
            rq = spool.tile([S, 1], f32, tag="srq", name="srq")
            nc.vector.reciprocal(rq[:], rs[:])
            prq = pspool.tile([1, SO], f32, tag="row", name="rqrow")
            nc.tensor.matmul(prq[:], rq[:], et, start=True, stop=True)
            rqr = spool.tile([1, SO], f32, tag="rqr", name="rqr")
            nc.vector.tensor_copy(rqr[:], prq[:])
            snr = []
            for dc in range(2):
                prep = pspool.tile([128, SO], f32, tag="big", name="snrep")
                nc.tensor.matmul(prep[:], st[:, dc * 128:(dc + 1) * 128],
                                 et, start=True, stop=True)
                sb = spool.tile([128, SO], f32, tag=f"snr{dc}",
                                name=f"snr{dc}")
                nc.vector.tensor_copy(sb[:], prep[:])
                snr.append(sb)

            ctp = [pppool.tile([128, SO], f32, tag=f"ctp{dc}", name=f"ctp{dc}")
                   for dc in range(2)]
            gev = ge.rearrange("(s r) d -> r s d", s=S)
            for grp in range(S // GRP):
                gt = gpool.tile([128, GRP * nch * 256], f32, tag="g", name="g")
                gtv = gt[:].rearrange("p (s c d) -> p s c d", s=GRP, c=nch)
                for ci in range(nch):
                    nc.sync.dma_start(
                        gtv[:, :, ci, :],
                        gev[ci * 128:(ci + 1) * 128,
                            grp * GRP:(grp + 1) * GRP, :])
                for si in range(GRP):
                    s = grp * GRP + si
                    for dc in range(2):
                        for ci in range(nch):
                            nc.tensor.matmul(
                                ctp[dc][:, s * K:(s + 1) * K],
                                gt[:, (si * nch + ci) * 256 + dc * 128:
                                   (si * nch + ci) * 256 + (dc + 1) * 128],
                                mtall[:, (s * nch + ci) * K:
                                      (s * nch + ci + 1) * K],
                                start=(ci == 0), stop=(ci == nch - 1))
            ct = []
            for dc in range(2):
                cb = apool.tile([128, SO], f32, tag=f"ct{dc}", name=f"ct{dc}")
                nc.vector.tensor_copy(cb[:], ctp[dc][:])
                ct.append(cb)

            ht = []
            for m in range(2):
                ph = pspool.tile([128, SO], f32, tag="big", name="mlp_h")
                nc.tensor.matmul(ph[:], w1t[0][:, m * 128:(m + 1) * 128],
                                 ct[0][:], start=True, stop=False)
                nc.tensor.matmul(ph[:], w1t[1][:, m * 128:(m + 1) * 128],
                                 ct[1][:], start=False, stop=False)
                nc.tensor.matmul(ph[:], w1t[2][:, m * 128:(m + 1) * 128],
                                 bbxt, start=False, stop=True)
                hb = apool.tile([128, SO], f32, tag=f"h{m}", name=f"h{m}")
                nc.scalar.activation(hb[:], ph[:], AF.Gelu, bias=bias_ap(0, m))
                ht.append(hb)

            objt = []
            for m in range(2):
                po = pspool.tile([128, SO], f32, tag="big", name="mlp_o")
                nc.tensor.matmul(po[:], w2t[0][:, m * 128:(m + 1) * 128],
                                 ht[0][:], start=True, stop=False)
                nc.tensor.matmul(po[:], w2t[1][:, m * 128:(m + 1) * 128],
                                 ht[1][:], start=False, stop=True)
                ob = apool.tile([128, SO], f32, tag=f"obj{m}", name=f"obj{m}")
                nc.vector.tensor_scalar_add(ob[:], po[:], bias_ap(1, m))
                nc.vector.tensor_mul(ob[:], ob[:], validb[:])
                objt.append(ob)

            pd = pspool.tile([1, SO], f32, tag="row", name="dot")
            for dc in range(2):
                tm = spool.tile([128, SO], f32, tag="dotmul", name="dotmul",
                                bufs=2)
                nc.vector.tensor_mul(tm[:], objt[dc][:], snr[dc][:])
                nc.tensor.matmul(pd[:], onescol[:], tm[:],
                                 start=(dc == 0), stop=(dc == 1))
            dotq = spool.tile([1, SO], f32, tag="dotq", name="dotq")
            nc.vector.tensor_mul(dotq[:], pd[:], rqr[:])
            dotb = bcast_row(dotq[:], "dot")
            cot = []
            for dc in range(2):
                cb = apool.tile([128, SO], f32, tag=f"co{dc}", name=f"co{dc}")
                nc.vector.tensor_mul(cb[:], dotb[:], snr[dc][:])
                nc.vector.tensor_sub(cb[:], objt[dc][:], cb[:])
                nc.vector.tensor_scalar_mul(cb[:], cb[:], orthob)
                cot.append(cb)

            cpt = []
            for m in range(2):
                pc = pspool.tile([128, SO], f32, tag="big", name="mlp_p")
                nc.tensor.matmul(pc[:], wpt[0][:, m * 128:(m + 1) * 128],
                                 cot[0][:], start=True, stop=False)
                nc.tensor.matmul(pc[:], wpt[1][:, m * 128:(m + 1) * 128],
                                 cot[1][:], start=False, stop=True)
                cb = apool.tile([128, SO], f32, tag=f"cp{m}", name=f"cp{m}")
                nc.vector.tensor_scalar_add(cb[:], pc[:], bias_ap(2, m))
                cpt.append(cb)

            ps1 = pspool.tile([1, SO], f32, tag="row", name="s1")
            ps2 = pspool.tile([1, SO], f32, tag="row", name="s2")
            for m in range(2):
                nc.tensor.matmul(ps1[:], onescol[:], cpt[m][:],
                                 start=(m == 0), stop=(m == 1))
            sqs = []
            for m in range(2):
                qq = spool.tile([128, SO], f32, tag="lnsq", name="lnsq",
                                bufs=2)
                nc.vector.tensor_mul(qq[:], cpt[m][:], cpt[m][:])
                sqs.append(qq)
            for m in range(2):
                nc.tensor.matmul(ps2[:], onescol[:], sqs[m][:],
                                 start=(m == 0), stop=(m == 1))
            mur = spool.tile([1, SO], f32, tag="mur", name="mur")
            nc.vector.tensor_scalar_mul(mur[:], ps1[:], 1.0 / D)
            msr = spool.tile([1, SO], f32, tag="msr", name="msr")
            nc.vector.tensor_scalar_mul(msr[:], ps2[:], 1.0 / D)
            varr = spool.tile([1, SO], f32, tag="varr", name="varr")
            nc.vector.tensor_mul(varr[:], mur[:], mur[:])
            nc.vector.tensor_sub(varr[:], msr[:], varr[:])
            nc.vector.tensor_scalar_add(varr[:], varr[:], 1e-5)
            nc.scalar.activation(varr[:], varr[:], AF.Sqrt)
            rstd = spool.tile([1, SO], f32, tag="rstd", name="rstd")
            nc.vector.reciprocal(rstd[:], varr[:])
            mub = bcast_row(mur[:], "mu")
            rstdb = bcast_row(rstd[:], "rstd")
            yt = []
            for m in range(2):
                yb = apool.tile([128, SO], f32, tag=f"y{m}", name=f"y{m}")
                nc.vector.tensor_sub(yb[:], cpt[m][:], mub[:])
                nc.vector.tensor_mul(yb[:], yb[:], rstdb[:])
                nc.vector.tensor_scalar(yb[:], yb[:], bias_ap(3, m),
                                        bias_ap(4, m), op0=MUL, op1=ADD)
                yt.append(yb)

            for q in range(2):
                yo = spool.tile([128, D], f32, tag="yo", name="yo", bufs=2)
                for m in range(2):
                    pt = pspool.tile([128, 128], f32, tag="big", name="tr")
                    nc.tensor.transpose(
                        pt[:], yt[m][:, q * 128:(q + 1) * 128], idt)
                    nc.vector.tensor_copy(yo[:, m * 128:(m + 1) * 128], pt[:])
                nc.sync.dma_start(out[q * 128:(q + 1) * 128, :], yo[:])

    nc.compile()
    return nc


def _get_program(rcap):
    if rcap not in _PROG:
        _PROG[rcap] = _build_program(rcap)
    return _PROG[rcap]



def _make_in_maps(np_inputs, mhat, bboxT, validf, rcap):
    grid_emb = np.asarray(np_inputs["grid_emb"], np.float32)
    structure_rep = np.asarray(np_inputs["structure_rep"], np.float32)
    W1 = np.asarray(np_inputs["W1"], np.float32)
    W2 = np.asarray(np_inputs["W2"], np.float32)
    Wp = np.asarray(np_inputs["Wp"], np.float32)
    b1 = np.asarray(np_inputs["b1"], np.float32)
    b2 = np.asarray(np_inputs["b2"], np.float32)
    bp = np.asarray(np_inputs["bp"], np.float32)
    gamma = np.asarray(np_inputs["gamma"], np.float32)
    beta = np.asarray(np_inputs["beta"], np.float32)
    orth = float(np.asarray(np_inputs["ortho_scale"]).reshape(-1)[0])

    wall = np.concatenate([W1[0:128], W1[128:256], W2[0:128], W2[128:256],
                           Wp[0:128], Wp[128:256]], axis=1)
    w1c = np.ascontiguousarray(W1[256:261])
    bpk = np.zeros((128, 11), np.float32)
    for m in range(2):
        for j, vec in enumerate((b1, b2, bp, gamma, beta)):
            bpk[:, m * 5 + j] = vec[m * 128:(m + 1) * 128]
    bpk[:, 10] = orth

    econ = np.zeros((S, SO), np.float32)
    for s in range(S):
        econ[s, s * K:(s + 1) * K] = 1.0
    selc = np.zeros((S * 8, S), np.float32)
    for s in range(S):
        selc[s * 8:(s + 1) * 8, s] = 1.0
    ident = np.eye(128, dtype=np.float32)

    in_maps = []
    for c in range(NCORES):
        sl = slice(c * S, (c + 1) * S)
        spkc = np.concatenate(
            [selc, structure_rep[sl].reshape(S * 8, D), ident], axis=1)
        epkc = np.zeros((S, 768), np.float32)
        epkc[:, 0:256] = econ
        epkc[0, 256:512] = validf[sl].reshape(SO)
        epkc[0:5, 512:768] = bboxT[sl].transpose(1, 0, 2).reshape(5, SO)
        in_maps.append(dict(
            ge=np.ascontiguousarray(grid_emb[sl].reshape(S * HW, D)),
            mh=np.ascontiguousarray(mhat[sl, :rcap].reshape(S * rcap, K)),
            wall=np.ascontiguousarray(wall),
            w1c=w1c, bpk=bpk,
            spk=np.ascontiguousarray(spkc),
            epk=epkc,
        ))
    return in_maps


def kernel(grid_emb, grid, structure_rep, W1, b1, W2, b2, Wp, bp,
           gamma, beta, ortho_scale):
    from concourse.bass_utils import run_bass_kernel_spmd

    np_inputs = dict(grid_emb=grid_emb, grid=grid,
                     structure_rep=structure_rep, W1=W1, b1=b1, W2=W2, b2=b2,
                     Wp=Wp, bp=bp, gamma=gamma, beta=beta,
                     ortho_scale=ortho_scale)
    grid = np.asarray(grid, np.int32)
    mhat, bboxT, validf = _build_masks(grid)
    rcap = _row_cap(mhat)
    in_maps = _make_in_maps(np_inputs, mhat, bboxT, validf, rcap)

    nc = _get_program(rcap)
    res = run_bass_kernel_spmd(nc, in_maps, list(range(NCORES)))
    outs = [res.results[c]["out"].reshape(S, K, D) for c in range(NCORES)]
    return np.concatenate(outs, axis=0)



# revision 6
# speedup vs baseline: 2.6736x; 2.6736x over previous
import sys

sys.path.insert(0, "/opt/trn_rl_repo")

import numpy as np
import ml_dtypes

BF16 = ml_dtypes.bfloat16

H, W = 30, 30
D = 256
K = 16
HW = H * W
SENT = HW
B = 128
NCORES = 8
S = B // NCORES
SO = S * K



def _label_components(grid):
    lin = np.arange(HW, dtype=np.int32).reshape(1, H, W)
    fg = grid > 0
    lab = np.where(fg, lin, SENT).astype(np.int32)
    gp = np.pad(grid, ((0, 0), (1, 1), (1, 1)), constant_values=-1)
    nb = grid.shape[0]
    while True:
        lp = np.pad(lab, ((0, 0), (1, 1), (1, 1)), constant_values=SENT)
        m = lab.copy()
        for di, dj in ((-1, 0), (1, 0), (0, -1), (0, 1)):
            ls = lp[:, 1 + di:1 + di + H, 1 + dj:1 + dj + W]
            gs = gp[:, 1 + di:1 + di + H, 1 + dj:1 + dj + W]
            m = np.minimum(m, np.where(gs == grid, ls, SENT))
        m = np.where(fg, m, SENT)
        flat = m.reshape(nb, HW)
        jumped = np.take_along_axis(flat, np.clip(flat, 0, HW - 1), axis=1)
        flat = np.where(flat < SENT, np.minimum(flat, jumped), SENT)
        new = flat.reshape(nb, H, W)
        if np.array_equal(new, lab):
            return new
        lab = new


def _extract(grid):
    nb = grid.shape[0]
    labels = _label_components(grid).reshape(nb, HW)
    gf = grid.reshape(nb, HW)
    lin = np.arange(HW, dtype=np.int32)
    rows, cols = lin // W, lin % W
    cells = []
    bboxT = np.zeros((nb, 5, K), np.float32)
    validf = np.zeros((nb, K), np.float32)
    for b in range(nb):
        l = labels[b]
        roots = np.nonzero((l == lin) & (l < SENT))[0][:K]
        wmap = {}
        for k, r in enumerate(roots):
            memb = l == r
            rs, cs = rows[memb], cols[memb]
            y, x = int(rs.min()), int(cs.min())
            h = int(rs.max()) + 1 - y
            w = int(cs.max()) + 1 - x
            wgt = 1.0 / float(h * w)
            for rr in range(y, y + h):
                base = rr * W
                for cc in range(x, x + w):
                    wmap.setdefault(base + cc, []).append((k, wgt))
            bboxT[b, :, k] = (gf[b, r] / 9.0, x / float(W), y / float(H),
                              w / float(W), h / float(H))
            validf[b, k] = 1.0
        cl = np.array(sorted(wmap), dtype=np.int64)
        wm = np.zeros((len(cl), K), np.float32)
        for i, c in enumerate(cl):
            for k, wgt in wmap[c]:
                wm[i, k] = wgt
        cells.append((cl, wm))
    return cells, bboxT, validf


def _prepare(np_inputs):
    grid = np.asarray(np_inputs["grid"], np.int32)
    grid_emb = np.asarray(np_inputs["grid_emb"], np.float32).reshape(B, HW, D)
    structure_rep = np.asarray(np_inputs["structure_rep"], np.float32)
    W1 = np.asarray(np_inputs["W1"], np.float32)
    W2 = np.asarray(np_inputs["W2"], np.float32)
    Wp = np.asarray(np_inputs["Wp"], np.float32)
    b1 = np.asarray(np_inputs["b1"], np.float32)
    b2 = np.asarray(np_inputs["b2"], np.float32)
    bp = np.asarray(np_inputs["bp"], np.float32)
    gamma = np.asarray(np_inputs["gamma"], np.float32)
    beta = np.asarray(np_inputs["beta"], np.float32)
    orth = float(np.asarray(np_inputs["ortho_scale"]).reshape(-1)[0])

    cells, bboxT, validf = _extract(grid)

    ncell_core = [sum(len(cells[c * S + s][0]) for s in range(S))
                  for c in range(NCORES)]
    nch = max(1, -(-max(ncell_core) // 128))

    has_bias = (np.any(b1) or np.any(b2) or np.any(bp))
    has_gb = (not np.all(gamma == 1.0)) or np.any(beta)
    need_valid = has_bias and not np.all(validf == 1.0)
    flags = (nch, bool(has_bias), bool(has_gb), bool(need_valid))

    Wpp = Wp * orth
    sm = structure_rep.mean(axis=1)
    sn = sm / np.maximum(np.linalg.norm(sm, axis=-1, keepdims=True), 1e-8)
    wsn = sn @ Wpp

    eye = np.eye(128, dtype=np.float32)
    et = np.zeros((S, SO), np.float32)
    for s in range(S):
        et[s, s * K:(s + 1) * K] = 1.0

    in_maps = []
    for c in range(NCORES):
        sl = slice(c * S, (c + 1) * S)
        gm = np.zeros((128, nch * 512), np.float32)
        i = 0
        for s in range(S):
            cl, wm = cells[c * S + s]
            for j in range(len(cl)):
                ch, row = divmod(i, 128)
                gm[row, ch * 512:ch * 512 + 256] = grid_emb[c * S + s, cl[j]]
                gm[row, ch * 512 + 256 + s * K:ch * 512 + 256 + (s + 1) * K] \
                    = wm[j]
                i += 1
        snF = np.zeros((128, 32), np.float32)
        for dc in range(2):
            snF[:, dc * 16:(dc + 1) * 16] = sn[sl][:, dc * 128:(dc + 1) * 128].T
        wall = np.concatenate(
            [W1[0:128], W1[128:256], W2[0:128], W2[128:256],
             Wpp[0:128], Wpp[128:256], snF, eye], axis=1)
        p5 = np.concatenate(
            [W1[256:261], bboxT[sl].transpose(1, 0, 2).reshape(5, SO)], axis=1)
        p16 = np.concatenate([et, -wsn[sl]], axis=1)

        im = dict(gm=gm.astype(BF16), wall=wall.astype(BF16),
                  p5=p5.astype(BF16), p16=p16.astype(BF16))
        if has_bias:
            bb = np.zeros((128, 6), np.float32)
            for m in range(2):
                bb[:, m] = b1[m * 128:(m + 1) * 128]
                bb[:, 2 + m] = b2[m * 128:(m + 1) * 128]
                bb[:, 4 + m] = bp[m * 128:(m + 1) * 128]
            im["biasp"] = bb
        if has_gb:
            gb = np.zeros((128, 512), np.float32)
            gb[:, 0:256] = gamma[None, :]
            gb[:, 256:512] = beta[None, :]
            im["gbp"] = gb.astype(BF16)
        if need_valid:
            im["vldp"] = np.broadcast_to(
                validf[sl].reshape(1, SO), (128, SO)).astype(BF16).copy()
        in_maps.append(im)
    return flags, in_maps



_PROG = {}


def _build_program(flags):
    import concourse.bacc as bacc
    import concourse.mybir as mybir
    import concourse.tile as tile

    nch, has_bias, has_gb, need_valid = flags
    f32 = mybir.dt.float32
    bf16 = mybir.dt.bfloat16
    AF = mybir.ActivationFunctionType
    MUL = mybir.AluOpType.mult
    SUB = mybir.AluOpType.subtract

    nc = bacc.Bacc("TRN2", target_bir_lowering=False, debug=False,
                   num_devices=NCORES)

    gmp = nc.declare_dram_parameter("gm", [128, nch * 512], bf16,
                                    isOutput=False)
    wallp = nc.declare_dram_parameter("wall", [128, 1696], bf16,
                                      isOutput=False)
    p5p = nc.declare_dram_parameter("p5", [5, 256 + SO], bf16, isOutput=False)
    p16p = nc.declare_dram_parameter("p16", [S, SO + D], bf16, isOutput=False)
    if has_bias:
        biasp = nc.declare_dram_parameter("biasp", [128, 6], f32,
                                          isOutput=False)
    if has_gb:
        gbp = nc.declare_dram_parameter("gbp", [128, 512], bf16,
                                        isOutput=False)
    if need_valid:
        vldp = nc.declare_dram_parameter("vldp", [128, SO], bf16,
                                         isOutput=False)
    out = nc.declare_dram_parameter("out", [SO, D], f32, isOutput=True)

    with tile.TileContext(nc) as tc:
        with (
            tc.tile_pool(name="const", bufs=1) as cpool,
            tc.tile_pool(name="gp", bufs=max(2, nch)) as gpool,
            tc.tile_pool(name="act", bufs=1) as apool,
            tc.tile_pool(name="scr", bufs=2) as spool,
            tc.tile_pool(name="ps", bufs=3, space="PSUM") as pspool,
            tc.tile_pool(name="pp", bufs=1, space="PSUM") as pppool,
            tc.tile_pool(name="pq", bufs=1, space="PSUM") as pqpool,
        ):
            wallt = cpool.tile([128, 1696], bf16, tag="wall", name="wall")
            nc.scalar.dma_start(wallt[:], wallp[:])
            p5t = cpool.tile([5, 256 + SO], bf16, tag="p5", name="p5")
            nc.scalar.dma_start(p5t[:], p5p[:])
            p16t = cpool.tile([S, SO + D], bf16, tag="p16", name="p16")
            nc.scalar.dma_start(p16t[:], p16p[:])
            if has_bias:
                biast = cpool.tile([128, 6], f32, tag="biasp", name="biasp")
                nc.scalar.dma_start(biast[:], biasp[:])
            if has_gb:
                gbt = cpool.tile([128, 512], bf16, tag="gbp", name="gbp")
                nc.scalar.dma_start(gbt[:], gbp[:])
            if need_valid:
                vldt = cpool.tile([128, SO], bf16, tag="vldp", name="vldp")
                nc.scalar.dma_start(vldt[:], vldp[:])

            cnst = cpool.tile([128, 2], f32, tag="cnst", name="cnst")
            nc.vector.memset(cnst[:, 0:1], 0.0)
            nc.vector.memset(cnst[:, 1:2], 1e-5)
            zb = cnst[:, 0:1]
            epsb = cnst[:, 1:2]

            w1t = [wallt[:, 0:256], wallt[:, 256:512]]
            w2t = [wallt[:, 512:768], wallt[:, 768:1024]]
            wpt = [wallt[:, 1024:1280], wallt[:, 1280:1536]]
            snf = wallt[:, 1536:1568]
            idt = wallt[:, 1568:1696]
            w1c = p5t[:, 0:256]
            bbxt = p5t[:, 256:256 + SO]
            ett = p16t[:, 0:SO]
            wsnt = p16t[:, SO:SO + D]

            poolall = pppool.tile([128, 2 * SO], f32, tag="pool", name="pool")
            gts = []
            for ci in range(nch):
                gt = gpool.tile([128, 512], bf16, tag="gm", name=f"gm{ci}")
                nc.sync.dma_start(gt[:], gmp[:, ci * 512:(ci + 1) * 512])
                gts.append(gt)
            for dc in range(2):
                for ci in range(nch):
                    nc.tensor.matmul(
                        poolall[:, dc * SO:(dc + 1) * SO],
                        gts[ci][:, dc * 128:(dc + 1) * 128],
                        gts[ci][:, 256:512],
                        start=(ci == 0), stop=(ci == nch - 1))
            cpl = []
            for dc in range(2):
                cb = apool.tile([128, SO], bf16, tag=f"cpl{dc}",
                                name=f"cpl{dc}")
                nc.scalar.activation(cb[:], poolall[:, dc * SO:(dc + 1) * SO],
                                     AF.Copy)
                cpl.append(cb)

            ht = []
            for m in range(2):
                ph = pspool.tile([128, SO], f32, tag="big", name="mlp_h")
                nc.tensor.matmul(ph[:], w1t[0][:, m * 128:(m + 1) * 128],
                                 cpl[0][:], start=True, stop=False)
                nc.tensor.matmul(ph[:], w1t[1][:, m * 128:(m + 1) * 128],
                                 cpl[1][:], start=False, stop=False)
                nc.tensor.matmul(ph[:], w1c[:, m * 128:(m + 1) * 128],
                                 bbxt, start=False, stop=True)
                hb = apool.tile([128, SO], bf16, tag=f"h{m}", name=f"h{m}")
                gb_ap = biast[:, m:m + 1] if has_bias else zb
                nc.scalar.activation(hb[:], ph[:], AF.Gelu, bias=gb_ap)
                ht.append(hb)

            objt = []
            for m in range(2):
                po = pspool.tile([128, SO], f32, tag="big", name="mlp_o")
                nc.tensor.matmul(po[:], w2t[0][:, m * 128:(m + 1) * 128],
                                 ht[0][:], start=True, stop=False)
                nc.tensor.matmul(po[:], w2t[1][:, m * 128:(m + 1) * 128],
                                 ht[1][:], start=False, stop=True)
                ob = apool.tile([128, SO], bf16, tag=f"obj{m}", name=f"obj{m}")
                if has_bias:
                    nc.vector.tensor_scalar_add(ob[:], po[:],
                                                biast[:, 2 + m:3 + m])
                    if need_valid:
                        nc.vector.tensor_mul(ob[:], ob[:], vldt[:])
                else:
                    nc.scalar.activation(ob[:], po[:], AF.Copy)
                objt.append(ob)

            dtps = pqpool.tile([S, SO], f32, tag="dt", name="dt")
            for dc in range(2):
                nc.tensor.matmul(dtps[:], snf[:, dc * 16:(dc + 1) * 16],
                                 objt[dc][:], start=(dc == 0), stop=(dc == 1))
            masked = spool.tile([S, SO], bf16, tag="masked", name="masked")
            nc.vector.tensor_mul(masked[:], dtps[:], ett)

            cpt = []
            for m in range(2):
                pc = pspool.tile([128, SO], f32, tag="big", name="mlp_p")
                nc.tensor.matmul(pc[:], wpt[0][:, m * 128:(m + 1) * 128],
                                 objt[0][:], start=True, stop=False)
                nc.tensor.matmul(pc[:], wpt[1][:, m * 128:(m + 1) * 128],
                                 objt[1][:], start=False, stop=False)
                nc.tensor.matmul(pc[:], wsnt[:, m * 128:(m + 1) * 128],
                                 masked[:], start=False, stop=True)
                cb = apool.tile([128, SO], bf16, tag=f"cp{m}", name=f"cp{m}")
                if has_bias:
                    nc.vector.tensor_scalar_add(cb[:], pc[:],
                                                biast[:, 4 + m:5 + m])
                else:
                    nc.scalar.activation(cb[:], pc[:], AF.Copy)
                cpt.append(cb)

            tall = pqpool.tile([128, 2 * D], bf16, tag="tall", name="tall")
            for q in range(2):
                tps = tall[:, q * D:(q + 1) * D]
                for m in range(2):
                    nc.tensor.transpose(tps[:, m * 128:(m + 1) * 128],
                                        cpt[m][:, q * 128:(q + 1) * 128], idt)
                musum = spool.tile([128, 1], f32, tag="musum", name="musum")
                nc.vector.reduce_sum(musum[:], tps,
                                     axis=mybir.AxisListType.X)
                sqscr = spool.tile([128, D], f32, tag="sqscr", name="sqscr")
                ssq = spool.tile([128, 1], f32, tag="ssq", name="ssq")
                nc.scalar.activation(sqscr[:], tps, AF.Square,
                                     bias=zb, accum_out=ssq[:])
                mut = spool.tile([128, 1], f32, tag="mut", name="mut")
                nc.vector.tensor_scalar_mul(mut[:], musum[:], 1.0 / D)
                varr = spool.tile([128, 1], f32, tag="varr", name="varr")
                nc.vector.tensor_scalar(varr[:], ssq[:], 1.0 / D, None,
                                        op0=MUL)
                muu = spool.tile([128, 1], f32, tag="muu", name="muu")
                nc.vector.tensor_scalar(muu[:], mut[:], mut[:, 0:1], None,
                                        op0=MUL)
                nc.vector.tensor_sub(varr[:], varr[:], muu[:])
                stdt = spool.tile([128, 1], f32, tag="stdt", name="stdt")
                nc.scalar.activation(stdt[:], varr[:], AF.Sqrt, bias=epsb)
                rstd = spool.tile([128, 1], f32, tag="rstd", name="rstd")
                nc.vector.reciprocal(rstd[:], stdt[:])
                ms = spool.tile([128, 1], f32, tag="ms", name="ms")
                nc.vector.tensor_scalar(ms[:], mut[:], rstd[:, 0:1], None,
                                        op0=MUL)
                yb = spool.tile([128, D], f32, tag="y", name=f"y{q}")
                nc.vector.tensor_scalar(yb[:], tps, rstd[:, 0:1],
                                        ms[:, 0:1], op0=MUL, op1=SUB)
                if has_gb:
                    nc.vector.tensor_mul(yb[:], yb[:], gbt[:, 0:256])
                    nc.vector.tensor_add(yb[:], yb[:], gbt[:, 256:512])
                nc.sync.dma_start(out[q * 128:(q + 1) * 128, :], yb[:])

    nc.compile()
    return nc


def _get_program(flags):
    if flags not in _PROG:
        _PROG[flags] = _build_program(flags)
    return _PROG[flags]



def kernel(grid_emb, grid, structure_rep, W1, b1, W2, b2, Wp, bp,
           gamma, beta, ortho_scale):
    from concourse.bass_utils import run_bass_kernel_spmd

    np_inputs = dict(grid_emb=grid_emb, grid=grid,
                     structure_rep=structure_rep, W1=W1, b1=b1, W2=W2, b2=b2,
                     Wp=Wp, bp=bp, gamma=gamma, beta=beta,
                     ortho_scale=ortho_scale)
    flags, in_maps = _prepare(np_inputs)
    nc = _get_program(flags)
    res = run_bass_kernel_spmd(nc, in_maps, list(range(NCORES)))
    outs = [res.results[c]["out"].reshape(S, K, D) for c in range(NCORES)]
    return np.concatenate(outs, axis=0)


# revision 11
# speedup vs baseline: 2.9949x; 1.1202x over previous
import sys

sys.path.insert(0, "/opt/trn_rl_repo")

import numpy as np
import ml_dtypes

BF16 = ml_dtypes.bfloat16

H, W = 30, 30
D = 256
K = 16
HW = H * W
SENT = HW
B = 128
NCORES = 8
S = B // NCORES
SO = S * K



def _label_components(grid):
    lin = np.arange(HW, dtype=np.int32).reshape(1, H, W)
    fg = grid > 0
    lab = np.where(fg, lin, SENT).astype(np.int32)
    gp = np.pad(grid, ((0, 0), (1, 1), (1, 1)), constant_values=-1)
    nb = grid.shape[0]
    while True:
        lp = np.pad(lab, ((0, 0), (1, 1), (1, 1)), constant_values=SENT)
        m = lab.copy()
        for di, dj in ((-1, 0), (1, 0), (0, -1), (0, 1)):
            ls = lp[:, 1 + di:1 + di + H, 1 + dj:1 + dj + W]
            gs = gp[:, 1 + di:1 + di + H, 1 + dj:1 + dj + W]
            m = np.minimum(m, np.where(gs == grid, ls, SENT))
        m = np.where(fg, m, SENT)
        flat = m.reshape(nb, HW)
        jumped = np.take_along_axis(flat, np.clip(flat, 0, HW - 1), axis=1)
        flat = np.where(flat < SENT, np.minimum(flat, jumped), SENT)
        new = flat.reshape(nb, H, W)
        if np.array_equal(new, lab):
            return new
        lab = new


def _extract(grid):
    nb = grid.shape[0]
    labels = _label_components(grid).reshape(nb, HW)
    gf = grid.reshape(nb, HW)
    lin = np.arange(HW, dtype=np.int32)
    rows, cols = lin // W, lin % W
    cells = []
    bboxT = np.zeros((nb, 5, K), np.float32)
    validf = np.zeros((nb, K), np.float32)
    for b in range(nb):
        l = labels[b]
        roots = np.nonzero((l == lin) & (l < SENT))[0][:K]
        wmap = {}
        for k, r in enumerate(roots):
            memb = l == r
            rs, cs = rows[memb], cols[memb]
            y, x = int(rs.min()), int(cs.min())
            h = int(rs.max()) + 1 - y
            w = int(cs.max()) + 1 - x
            wgt = 1.0 / float(h * w)
            for rr in range(y, y + h):
                base = rr * W
                for cc in range(x, x + w):
                    wmap.setdefault(base + cc, []).append((k, wgt))
            bboxT[b, :, k] = (gf[b, r] / 9.0, x / float(W), y / float(H),
                              w / float(W), h / float(H))
            validf[b, k] = 1.0
        cl = np.array(sorted(wmap), dtype=np.int64)
        wm = np.zeros((len(cl), K), np.float32)
        for i, c in enumerate(cl):
            for k, wgt in wmap[c]:
                wm[i, k] = wgt
        cells.append((cl, wm))
    return cells, bboxT, validf


def _prepare_gen(np_inputs, pre):
    grid = np.asarray(np_inputs["grid"], np.int32)
    grid_emb = np.asarray(np_inputs["grid_emb"], np.float32).reshape(B, HW, D)
    structure_rep = np.asarray(np_inputs["structure_rep"], np.float32)
    W1 = np.asarray(np_inputs["W1"], np.float32)
    W2 = np.asarray(np_inputs["W2"], np.float32)
    Wp = np.asarray(np_inputs["Wp"], np.float32)
    b1 = np.asarray(np_inputs["b1"], np.float32)
    b2 = np.asarray(np_inputs["b2"], np.float32)
    bp = np.asarray(np_inputs["bp"], np.float32)
    gamma = np.asarray(np_inputs["gamma"], np.float32)
    beta = np.asarray(np_inputs["beta"], np.float32)
    orth = float(np.asarray(np_inputs["ortho_scale"]).reshape(-1)[0])

    cells, bboxT, validf = pre

    ncell_core = [sum(len(cells[c * S + s][0]) for s in range(S))
                  for c in range(NCORES)]
    nch = max(1, -(-max(ncell_core) // 128))

    has_bias = (np.any(b1) or np.any(b2) or np.any(bp))
    has_gb = (not np.all(gamma == 1.0)) or np.any(beta)
    need_valid = has_bias and not np.all(validf == 1.0)
    flags = ("gen", nch, bool(has_bias), bool(has_gb), bool(need_valid))

    Wpp = Wp * orth
    sm = structure_rep.mean(axis=1)
    sn = sm / np.maximum(np.linalg.norm(sm, axis=-1, keepdims=True), 1e-8)
    wsn = sn @ Wpp

    eye = np.eye(128, dtype=np.float32)
    et = np.zeros((S, SO), np.float32)
    for s in range(S):
        et[s, s * K:(s + 1) * K] = 1.0

    in_maps = []
    for c in range(NCORES):
        sl = slice(c * S, (c + 1) * S)
        gm = np.zeros((128, nch * 512), np.float32)
        i = 0
        for s in range(S):
            cl, wm = cells[c * S + s]
            for j in range(len(cl)):
                ch, row = divmod(i, 128)
                gm[row, ch * 512:ch * 512 + 256] = grid_emb[c * S + s, cl[j]]
                gm[row, ch * 512 + 256 + s * K:ch * 512 + 256 + (s + 1) * K] \
                    = wm[j]
                i += 1
        snF = np.zeros((128, 32), np.float32)
        for dc in range(2):
            snF[:, dc * 16:(dc + 1) * 16] = sn[sl][:, dc * 128:(dc + 1) * 128].T
        wall = np.concatenate(
            [W1[0:128], W1[128:256], W2[0:128], W2[128:256],
             Wpp[0:128], Wpp[128:256], snF, eye], axis=1)
        p5 = np.concatenate(
            [W1[256:261], bboxT[sl].transpose(1, 0, 2).reshape(5, SO)], axis=1)
        p16 = np.concatenate([et, -wsn[sl]], axis=1)

        im = dict(gm=gm.astype(BF16), wall=wall.astype(BF16),
                  p5=p5.astype(BF16), p16=p16.astype(BF16))
        if has_bias:
            bb = np.zeros((128, 6), np.float32)
            for m in range(2):
                bb[:, m] = b1[m * 128:(m + 1) * 128]
                bb[:, 2 + m] = b2[m * 128:(m + 1) * 128]
                bb[:, 4 + m] = bp[m * 128:(m + 1) * 128]
            im["biasp"] = bb
        if has_gb:
            gb = np.zeros((128, 512), np.float32)
            gb[:, 0:256] = gamma[None, :]
            gb[:, 256:512] = beta[None, :]
            im["gbp"] = gb.astype(BF16)
        if need_valid:
            im["vldp"] = np.broadcast_to(
                validf[sl].reshape(1, SO), (128, SO)).astype(BF16).copy()
        in_maps.append(im)
    return flags, in_maps



_PROG = {}


def _build_gen(flags):
    import concourse.bacc as bacc
    import concourse.mybir as mybir
    import concourse.tile as tile

    _, nch, has_bias, has_gb, need_valid = flags
    f32 = mybir.dt.float32
    bf16 = mybir.dt.bfloat16
    AF = mybir.ActivationFunctionType
    MUL = mybir.AluOpType.mult
    SUB = mybir.AluOpType.subtract

    nc = bacc.Bacc("TRN2", target_bir_lowering=False, debug=False,
                   num_devices=NCORES)

    gmp = nc.declare_dram_parameter("gm", [128, nch * 512], bf16,
                                    isOutput=False)
    wallp = nc.declare_dram_parameter("wall", [128, 1696], bf16,
                                      isOutput=False)
    p5p = nc.declare_dram_parameter("p5", [5, 256 + SO], bf16, isOutput=False)
    p16p = nc.declare_dram_parameter("p16", [S, SO + D], bf16, isOutput=False)
    if has_bias:
        biasp = nc.declare_dram_parameter("biasp", [128, 6], f32,
                                          isOutput=False)
    if has_gb:
        gbp = nc.declare_dram_parameter("gbp", [128, 512], bf16,
                                        isOutput=False)
    if need_valid:
        vldp = nc.declare_dram_parameter("vldp", [128, SO], bf16,
                                         isOutput=False)
    out = nc.declare_dram_parameter("out", [SO, D], f32, isOutput=True)

    with tile.TileContext(nc) as tc:
        with (
            tc.tile_pool(name="const", bufs=1) as cpool,
            tc.tile_pool(name="gp", bufs=max(2, nch)) as gpool,
            tc.tile_pool(name="act", bufs=1) as apool,
            tc.tile_pool(name="scr", bufs=2) as spool,
            tc.tile_pool(name="ps", bufs=3, space="PSUM") as pspool,
            tc.tile_pool(name="pp", bufs=1, space="PSUM") as pppool,
            tc.tile_pool(name="pq", bufs=1, space="PSUM") as pqpool,
        ):
            wallt = cpool.tile([128, 1696], bf16, tag="wall", name="wall")
            nc.scalar.dma_start(wallt[:], wallp[:])
            p5t = cpool.tile([5, 256 + SO], bf16, tag="p5", name="p5")
            nc.scalar.dma_start(p5t[:], p5p[:])
            p16t = cpool.tile([S, SO + D], bf16, tag="p16", name="p16")
            nc.scalar.dma_start(p16t[:], p16p[:])
            if has_bias:
                biast = cpool.tile([128, 6], f32, tag="biasp", name="biasp")
                nc.scalar.dma_start(biast[:], biasp[:])
            if has_gb:
                gbt = cpool.tile([128, 512], bf16, tag="gbp", name="gbp")
                nc.scalar.dma_start(gbt[:], gbp[:])
            if need_valid:
                vldt = cpool.tile([128, SO], bf16, tag="vldp", name="vldp")
                nc.scalar.dma_start(vldt[:], vldp[:])

            cnst = cpool.tile([128, 2], f32, tag="cnst", name="cnst")
            nc.vector.memset(cnst[:, 0:1], 0.0)
            nc.vector.memset(cnst[:, 1:2], 1e-5)
            zb = cnst[:, 0:1]
            epsb = cnst[:, 1:2]

            w1t = [wallt[:, 0:256], wallt[:, 256:512]]
            w2t = [wallt[:, 512:768], wallt[:, 768:1024]]
            wpt = [wallt[:, 1024:1280], wallt[:, 1280:1536]]
            snf = wallt[:, 1536:1568]
            idt = wallt[:, 1568:1696]
            w1c = p5t[:, 0:256]
            bbxt = p5t[:, 256:256 + SO]
            ett = p16t[:, 0:SO]
            wsnt = p16t[:, SO:SO + D]

            poolall = pppool.tile([128, 2 * SO], f32, tag="pool", name="pool")
            gts = []
            for ci in range(nch):
                gt = gpool.tile([128, 512], bf16, tag="gm", name=f"gm{ci}")
                nc.sync.dma_start(gt[:], gmp[:, ci * 512:(ci + 1) * 512])
                gts.append(gt)
            for dc in range(2):
                for ci in range(nch):
                    nc.tensor.matmul(
                        poolall[:, dc * SO:(dc + 1) * SO],
                        gts[ci][:, dc * 128:(dc + 1) * 128],
                        gts[ci][:, 256:512],
                        start=(ci == 0), stop=(ci == nch - 1))
            cpl = []
            for dc in range(2):
                cb = apool.tile([128, SO], bf16, tag=f"cpl{dc}",
                                name=f"cpl{dc}")
                nc.scalar.activation(cb[:], poolall[:, dc * SO:(dc + 1) * SO],
                                     AF.Copy)
                cpl.append(cb)

            ht = []
            for m in range(2):
                ph = pspool.tile([128, SO], f32, tag="big", name="mlp_h")
                nc.tensor.matmul(ph[:], w1t[0][:, m * 128:(m + 1) * 128],
                                 cpl[0][:], start=True, stop=False)
                nc.tensor.matmul(ph[:], w1t[1][:, m * 128:(m + 1) * 128],
                                 cpl[1][:], start=False, stop=False)
                nc.tensor.matmul(ph[:], w1c[:, m * 128:(m + 1) * 128],
                                 bbxt, start=False, stop=True)
                hb = apool.tile([128, SO], bf16, tag=f"h{m}", name=f"h{m}")
                gb_ap = biast[:, m:m + 1] if has_bias else zb
                nc.scalar.activation(hb[:], ph[:], AF.Gelu, bias=gb_ap)
                ht.append(hb)

            objt = []
            for m in range(2):
                po = pspool.tile([128, SO], f32, tag="big", name="mlp_o")
                nc.tensor.matmul(po[:], w2t[0][:, m * 128:(m + 1) * 128],
                                 ht[0][:], start=True, stop=False)
                nc.tensor.matmul(po[:], w2t[1][:, m * 128:(m + 1) * 128],
                                 ht[1][:], start=False, stop=True)
                ob = apool.tile([128, SO], bf16, tag=f"obj{m}", name=f"obj{m}")
                if has_bias:
                    nc.vector.tensor_scalar_add(ob[:], po[:],
                                                biast[:, 2 + m:3 + m])
                    if need_valid:
                        nc.vector.tensor_mul(ob[:], ob[:], vldt[:])
                else:
                    nc.scalar.activation(ob[:], po[:], AF.Copy)
                objt.append(ob)

            dtps = pqpool.tile([S, SO], f32, tag="dt", name="dt")
            for dc in range(2):
                nc.tensor.matmul(dtps[:], snf[:, dc * 16:(dc + 1) * 16],
                                 objt[dc][:], start=(dc == 0), stop=(dc == 1))
            masked = spool.tile([S, SO], bf16, tag="masked", name="masked")
            nc.vector.tensor_mul(masked[:], dtps[:], ett)

            cpt = []
            for m in range(2):
                pc = pspool.tile([128, SO], f32, tag="big", name="mlp_p")
                nc.tensor.matmul(pc[:], wpt[0][:, m * 128:(m + 1) * 128],
                                 objt[0][:], start=True, stop=False)
                nc.tensor.matmul(pc[:], wpt[1][:, m * 128:(m + 1) * 128],
                                 objt[1][:], start=False, stop=False)
                nc.tensor.matmul(pc[:], wsnt[:, m * 128:(m + 1) * 128],
                                 masked[:], start=False, stop=True)
                cb = apool.tile([128, SO], bf16, tag=f"cp{m}", name=f"cp{m}")
                if has_bias:
                    nc.vector.tensor_scalar_add(cb[:], pc[:],
                                                biast[:, 4 + m:5 + m])
                else:
                    nc.scalar.activation(cb[:], pc[:], AF.Copy)
                cpt.append(cb)

            tall = pqpool.tile([128, 2 * D], bf16, tag="tall", name="tall")
            for q in range(2):
                tps = tall[:, q * D:(q + 1) * D]
                for m in range(2):
                    nc.tensor.transpose(tps[:, m * 128:(m + 1) * 128],
                                        cpt[m][:, q * 128:(q + 1) * 128], idt)
                musum = spool.tile([128, 1], f32, tag="musum", name="musum")
                nc.vector.reduce_sum(musum[:], tps,
                                     axis=mybir.AxisListType.X)
                sqscr = spool.tile([128, D], f32, tag="sqscr", name="sqscr")
                ssq = spool.tile([128, 1], f32, tag="ssq", name="ssq")
                nc.scalar.activation(sqscr[:], tps, AF.Square,
                                     bias=zb, accum_out=ssq[:])
                mut = spool.tile([128, 1], f32, tag="mut", name="mut")
                nc.vector.tensor_scalar_mul(mut[:], musum[:], 1.0 / D)
                varr = spool.tile([128, 1], f32, tag="varr", name="varr")
                nc.vector.tensor_scalar(varr[:], ssq[:], 1.0 / D, None,
                                        op0=MUL)
                muu = spool.tile([128, 1], f32, tag="muu", name="muu")
                nc.vector.tensor_scalar(muu[:], mut[:], mut[:, 0:1], None,
                                        op0=MUL)
                nc.vector.tensor_sub(varr[:], varr[:], muu[:])
                stdt = spool.tile([128, 1], f32, tag="stdt", name="stdt")
                nc.scalar.activation(stdt[:], varr[:], AF.Sqrt, bias=epsb)
                rstd = spool.tile([128, 1], f32, tag="rstd", name="rstd")
                nc.vector.reciprocal(rstd[:], stdt[:])
                ms = spool.tile([128, 1], f32, tag="ms", name="ms")
                nc.vector.tensor_scalar(ms[:], mut[:], rstd[:, 0:1], None,
                                        op0=MUL)
                yb = spool.tile([128, D], f32, tag="y", name=f"y{q}")
                nc.vector.tensor_scalar(yb[:], tps, rstd[:, 0:1],
                                        ms[:, 0:1], op0=MUL, op1=SUB)
                if has_gb:
                    nc.vector.tensor_mul(yb[:], yb[:], gbt[:, 0:256])
                    nc.vector.tensor_add(yb[:], yb[:], gbt[:, 256:512])
                nc.sync.dma_start(out[q * 128:(q + 1) * 128, :], yb[:])

    nc.compile()
    return nc


def _get_program(flags):
    if flags not in _PROG:
        builder = _build_fast if flags[0] == "fast" else _build_gen
        _PROG[flags] = builder(flags)
    return _PROG[flags]





def _fast_bounds(cells):
    counts = [[len(cells[c * S + s][0]) for s in range(S)]
              for c in range(NCORES)]
    bounds = []
    lo = 0
    while lo < S:
        hi = lo + 1
        while hi < S and max(sum(counts[c][lo:hi + 1])
                             for c in range(NCORES)) <= 128:
            hi += 1
        if max(sum(counts[c][lo:hi]) for c in range(NCORES)) > 128:
            return None
        bounds.append((lo, hi))
        lo = hi
    return tuple(bounds)


def _prepare_fast(np_inputs, pre, bounds):
    grid_emb = np.asarray(np_inputs["grid_emb"], np.float32).reshape(B, HW, D)
    W1 = np.asarray(np_inputs["W1"], np.float32)
    W2 = np.asarray(np_inputs["W2"], np.float32)
    Wp = np.asarray(np_inputs["Wp"], np.float32)
    structure_rep = np.asarray(np_inputs["structure_rep"], np.float32)
    orth = float(np.asarray(np_inputs["ortho_scale"]).reshape(-1)[0])
    cells, bboxT, validf = pre

    Wpp = Wp * orth
    Wfold = W2 @ Wpp
    sm = structure_rep.mean(axis=1)
    sn = sm / np.maximum(np.linalg.norm(sm, axis=-1, keepdims=True), 1e-8)
    wsn = sn @ Wpp
    snW2 = sn @ W2.T

    widths = [16 * (hi - lo) for lo, hi in bounds]
    offs = np.cumsum([0] + [256 + w for w in widths]).tolist()
    gmw = offs[-1]

    et = np.zeros((S, SO), np.float32)
    for s in range(S):
        et[s, s * K:(s + 1) * K] = 1.0
    eye = np.eye(128, dtype=np.float32)

    in_maps = []
    for c in range(NCORES):
        sl = slice(c * S, (c + 1) * S)
        gm = np.zeros((128, gmw), np.float32)
        for ci, (lo, hi) in enumerate(bounds):
            i = 0
            for s in range(lo, hi):
                cl, wm = cells[c * S + s]
                n = len(cl)
                gm[i:i + n, offs[ci]:offs[ci] + 256] = \
                    grid_emb[c * S + s, cl]
                gm[i:i + n, offs[ci] + 256 + (s - lo) * K:
                   offs[ci] + 256 + (s - lo + 1) * K] = wm
                i += n
        snF2 = np.zeros((128, 32), np.float32)
        snFp = np.zeros((128, 32), np.float32)
        for dc in range(2):
            snF2[:, dc * 16:(dc + 1) * 16] = \
                snW2[sl][:, dc * 128:(dc + 1) * 128].T
            snFp[:, dc * 16:(dc + 1) * 16] = \
                sn[sl][:, dc * 128:(dc + 1) * 128].T
        bbx = bboxT[sl] * validf[sl][:, None, :]
        p5 = np.zeros((128, 512), np.float32)
        p5[0:5, 0:256] = W1[256:261]
        p5[0:5, 256:512] = bbx.transpose(1, 0, 2).reshape(5, SO)
        p16 = np.zeros((128, 512), np.float32)
        p16[0:S, 0:256] = et
        p16[0:S, 256:512] = -wsn[sl]
        ca = np.concatenate([W1[0:128], W1[128:256], p5], axis=1)
        cb = np.concatenate([Wfold[0:128], Wfold[128:256], snF2, eye, p16],
                            axis=1)
        in_maps.append(dict(gm=gm.astype(BF16), ca=ca.astype(BF16),
                            cb=cb.astype(BF16)))
    return ("fast", bounds), in_maps


def _build_fast(flags):
    import concourse.bacc as bacc
    import concourse.mybir as mybir
    import concourse.tile as tile

    bounds = flags[1]
    widths = [16 * (hi - lo) for lo, hi in bounds]
    offs = np.cumsum([0] + [256 + w for w in widths]).tolist()
    gmw = offs[-1]

    f32 = mybir.dt.float32
    bf16 = mybir.dt.bfloat16
    AF = mybir.ActivationFunctionType
    MUL = mybir.AluOpType.mult
    SUB = mybir.AluOpType.subtract

    nc = bacc.Bacc("TRN2", target_bir_lowering=False, debug=False,
                   num_devices=NCORES)
    gmp = nc.declare_dram_parameter("gm", [128, gmw], bf16, isOutput=False)
    cap = nc.declare_dram_parameter("ca", [128, 1024], bf16, isOutput=False)
    cbp = nc.declare_dram_parameter("cb", [128, 1184], bf16, isOutput=False)
    out = nc.declare_dram_parameter("out", [SO, D], f32, isOutput=True)

    with tile.TileContext(nc) as tc:
        with (
            tc.tile_pool(name="const", bufs=1) as cpool,
            tc.tile_pool(name="scr", bufs=1) as spool,
            tc.tile_pool(name="ps", bufs=2, space="PSUM") as pspool,
            tc.tile_pool(name="pp", bufs=1, space="PSUM") as pppool,
        ):
            cnst = cpool.tile([128, 2], f32, tag="cnst", name="cnst")
            nc.vector.memset(cnst[:, 0:1], 0.0)
            nc.vector.memset(cnst[:, 1:2], 1e-5)
            zb = cnst[:, 0:1]
            epsb = cnst[:, 1:2]
            dscr = cpool.tile([128, 1], f32, tag="dscr", name="dscr")

            for fn in (AF.Gelu, AF.Square, AF.Sqrt):
                nc.scalar.activation(dscr[0:1, 0:1], cnst[0:1, 0:1], fn,
                                     bias=zb[0:1, 0:1])

            gmt = cpool.tile([128, gmw], bf16, tag="gm", name="gm")
            nc.sync.dma_start(gmt[:], gmp[:])
            cat = cpool.tile([128, 1024], bf16, tag="ca", name="ca")
            nc.scalar.dma_start(cat[:], cap[:])
            cbt = cpool.tile([128, 1184], bf16, tag="cb", name="cb")
            nc.scalar.dma_start(cbt[:], cbp[:])

            w1t = [cat[:, 0:256], cat[:, 256:512]]
            w1c = cat[0:5, 512:768]
            bbxt = cat[0:5, 768:1024]
            wft = [cbt[:, 0:256], cbt[:, 256:512]]
            snf2 = cbt[:, 512:544]
            idt = cbt[:, 544:672]
            ett = cbt[0:S, 672:928]
            wsnt = cbt[0:S, 928:1184]

            poolall = pppool.tile([128, 2 * SO], f32, tag="pool", name="pool")
            for dc in range(2):
                for ci, (lo, hi) in enumerate(bounds):
                    nc.tensor.matmul(
                        poolall[:, dc * SO + lo * K:dc * SO + hi * K],
                        gmt[:, offs[ci] + dc * 128:offs[ci] + (dc + 1) * 128],
                        gmt[:, offs[ci] + 256:offs[ci] + 256 + widths[ci]],
                        start=True, stop=True)
            cpl = spool.tile([128, 2 * SO], bf16, tag="cpl", name="cpl")
            nc.vector.tensor_copy(cpl[:], poolall[:])

            hps = pspool.tile([128, 2 * SO], f32, tag="hps", name="hps")
            for m in range(2):
                ph = hps[:, m * SO:(m + 1) * SO]
                nc.tensor.matmul(ph, w1t[0][:, m * 128:(m + 1) * 128],
                                 cpl[:, 0:SO], start=True, stop=False)
                nc.tensor.matmul(ph, w1t[1][:, m * 128:(m + 1) * 128],
                                 cpl[:, SO:2 * SO], start=False, stop=False)
                nc.tensor.matmul(ph, w1c[:, m * 128:(m + 1) * 128],
                                 bbxt, start=False, stop=True)
            hall = spool.tile([128, 2 * SO], bf16, tag="hall", name="hall")
            nc.scalar.activation(hall[:], hps[:], AF.Gelu, bias=zb)
            ht = [hall[:, 0:SO], hall[:, SO:2 * SO]]

            dtps = pppool.tile([S, SO], f32, tag="dt", name="dt")
            for dc in range(2):
                nc.tensor.matmul(dtps[:], snf2[:, dc * 16:(dc + 1) * 16],
                                 ht[dc], start=(dc == 0), stop=(dc == 1))
            masked = spool.tile([S, SO], bf16, tag="masked", name="masked")
            nc.vector.tensor_mul(masked[:], dtps[:], ett)

            cps = pspool.tile([128, 2 * SO], f32, tag="cps", name="cps")
            for m in range(2):
                pc = cps[:, m * SO:(m + 1) * SO]
                nc.tensor.matmul(pc, wft[0][:, m * 128:(m + 1) * 128],
                                 ht[0], start=True, stop=False)
                nc.tensor.matmul(pc, wft[1][:, m * 128:(m + 1) * 128],
                                 ht[1], start=False, stop=False)
                nc.tensor.matmul(pc, wsnt[:, m * 128:(m + 1) * 128],
                                 masked[:], start=False, stop=True)
            cpt = spool.tile([128, 2 * SO], bf16, tag="cpt", name="cpt")
            nc.vector.tensor_copy(cpt[:], cps[:])

            tall = pppool.tile([128, 2 * D], bf16, tag="tall", name="tall")
            for q in range(2):
                for m in range(2):
                    nc.tensor.transpose(
                        tall[:, q * D + m * 128:q * D + (m + 1) * 128],
                        cpt[:, m * SO + q * 128:m * SO + (q + 1) * 128], idt)

            asum = spool.tile([128, 2], f32, tag="asum", name="asum")
            nc.vector.reduce_sum(
                asum[:], tall[:].rearrange("p (q d) -> p q d", q=2),
                axis=mybir.AxisListType.X)
            sqf = spool.tile([128, 2 * D], f32, tag="sqf", name="sqf")
            qsum = spool.tile([128, 2], f32, tag="qsum", name="qsum")
            for q in range(2):
                nc.scalar.activation(sqf[:, q * D:(q + 1) * D],
                                     tall[:, q * D:(q + 1) * D], AF.Square,
                                     bias=zb, accum_out=qsum[:, q:q + 1])
            am = spool.tile([128, 2], f32, tag="am", name="am")
            nc.vector.tensor_scalar(am[:], asum[:], 1.0 / D, None, op0=MUL)
            muu = spool.tile([128, 2], f32, tag="muu", name="muu")
            nc.vector.tensor_mul(muu[:], am[:], am[:])
            varr = spool.tile([128, 2], f32, tag="varr", name="varr")
            nc.vector.tensor_scalar(varr[:], qsum[:], 1.0 / D, None, op0=MUL)
            nc.vector.tensor_sub(varr[:], varr[:], muu[:])
            stdt = spool.tile([128, 2], f32, tag="stdt", name="stdt")
            nc.scalar.activation(stdt[:], varr[:], AF.Sqrt, bias=epsb)
            rstd = spool.tile([128, 2], f32, tag="rstd", name="rstd")
            nc.vector.reciprocal(rstd[:], stdt[:])
            mst = spool.tile([128, 2], f32, tag="mst", name="mst")
            nc.vector.tensor_mul(mst[:], am[:], rstd[:])
            yall = spool.tile([128, 2 * D], f32, tag="yall", name="yall")
            for q in range(2):
                nc.vector.tensor_scalar(
                    yall[:, q * D:(q + 1) * D], tall[:, q * D:(q + 1) * D],
                    rstd[:, q:q + 1], mst[:, q:q + 1], op0=MUL, op1=SUB)
            nc.scalar.dma_start(
                out.rearrange("(q p) d -> p q d", q=2, p=128),
                yall[:].rearrange("p (q d) -> p q d", q=2))

    nc.compile()
    return nc


def _prepare(np_inputs):
    grid = np.asarray(np_inputs["grid"], np.int32)
    pre = _extract(grid)
    b1 = np.asarray(np_inputs["b1"], np.float32)
    b2 = np.asarray(np_inputs["b2"], np.float32)
    bp = np.asarray(np_inputs["bp"], np.float32)
    gamma = np.asarray(np_inputs["gamma"], np.float32)
    beta = np.asarray(np_inputs["beta"], np.float32)
    fast_ok = not (np.any(b1) or np.any(b2) or np.any(bp) or np.any(beta)
                   or not np.all(gamma == 1.0))
    if fast_ok:
        bounds = _fast_bounds(pre[0])
        if bounds is not None:
            return _prepare_fast(np_inputs, pre, bounds)
    return _prepare_gen(np_inputs, pre)



def kernel(grid_emb, grid, structure_rep, W1, b1, W2, b2, Wp, bp,
           gamma, beta, ortho_scale):
    from concourse.bass_utils import run_bass_kernel_spmd

    np_inputs = dict(grid_emb=grid_emb, grid=grid,
                     structure_rep=structure_rep, W1=W1, b1=b1, W2=W2, b2=b2,
                     Wp=Wp, bp=bp, gamma=gamma, beta=beta,
                     ortho_scale=ortho_scale)
    flags, in_maps = _prepare(np_inputs)
    nc = _get_program(flags)
    res = run_bass_kernel_spmd(nc, in_maps, list(range(NCORES)))
    outs = [res.results[c]["out"].reshape(S, K, D) for c in range(NCORES)]
    return np.concatenate(outs, axis=0)
